# revision 1
# baseline (speedup 1.0000x reference)
"""Paged-attention decode kernel for 8 TRN2 NeuronCores.

Data-parallel over sequences: core i owns sequences [8i, 8i+8). All host-side
index logic (block-table gather, slot_mapping scatter, context_len masking)
is folded into the per-core input layouts; the device kernel is a dense
  scores^T = K^T_chunk.T @ q     (per 128-key chunk, PSUM f32)
  e = exp(SCALE * scores^T)      (ACT, no max-subtraction needed: |s|~O(5))
  out_aug = e.T @ [V | valid]    (PV accumulated over chunks; col 128 = denom)
  out = out_aug[:, :128] / out_aug[:, 128]
pipeline. Masking rides on V: rows >= context_len are zeroed and their valid
column is 0, so both numerator and denominator only see valid keys.

The kernel is HBM-bandwidth bound (streams the whole KV working set once),
so K/V are shipped in reduced precision: V in bf16, K in fp8-e3m4 when every
context is near-full (long-softmax averaging keeps the quantization noise
far below the accuracy gate); otherwise K falls back to bf16.
"""

from contextlib import ExitStack

import numpy as np
import ml_dtypes

import concourse.bass as bass  # noqa: F401
import concourse.mybir as mybir
import concourse.tile as tile
from concourse import bacc
from concourse.bass_utils import run_bass_kernel_spmd

# ---- problem constants (hardcoded from the spec) ----
NUM_HEADS = 32
NUM_KV_HEADS = 8
HEAD_DIM = 128
SCALE = 0.08838834764831845  # 1/sqrt(128)
BATCH = 64
BLOCK_SIZE = 256
BLOCKS_PER_SEQ = 16
CTX = BLOCKS_PER_SEQ * BLOCK_SIZE  # 4096

N_CORES = 8
SEQ_PER_CORE = BATCH // N_CORES          # 8
GQ = NUM_HEADS // NUM_KV_HEADS           # 4 query heads per kv head
GROUPS = SEQ_PER_CORE * NUM_KV_HEADS     # 64 (seq, kvh) groups per core
NCHUNK = CTX // 128                      # 32 key chunks of 128
VW = HEAD_DIM + 1                        # V columns + valid column

DT = mybir.dt.bfloat16
NP_DT = ml_dtypes.bfloat16
# fp8-e3m4 K (|k| <= ~6 fits the +-15.5 range) cuts 24% of HBM traffic;
# scores accumulate in f32 so only K's mantissa rounding is lost. Only used
# when all contexts are near-full (see kernel()).
FP8_MIN_CTX = 3072

_NC_CACHE = {}


def build_nc(seq_per_core=SEQ_PER_CORE, nchunk=NCHUNK, kv_heads=NUM_KV_HEADS,
             k_fp8=True):
    """Build the per-core Bass graph (SPMD: same graph on all cores)."""
    groups = seq_per_core * kv_heads
    ctx_len = nchunk * 128
    kdt = mybir.dt.float8e3 if k_fp8 else DT
    nc = bacc.Bacc()
    kt_ext = nc.declare_dram_parameter(
        "kt", [seq_per_core, kv_heads, HEAD_DIM, ctx_len], kdt, isOutput=False
    )
    v_ext = nc.declare_dram_parameter(
        "vv", [seq_per_core, kv_heads, 128, nchunk, VW], DT, isOutput=False
    )
    q_ext = nc.declare_dram_parameter(
        "qt", [HEAD_DIM, groups * GQ], DT, isOutput=False
    )
    out_ext = nc.declare_dram_parameter(
        "out", [groups * GQ, HEAD_DIM], mybir.dt.float32, isOutput=True
    )

    f32 = mybir.dt.float32

    with tile.TileContext(nc) as tc, ExitStack() as ctx:
        qpool = ctx.enter_context(tc.tile_pool(name="qp", bufs=1))
        nbuf = 14 if k_fp8 else 8
        kpool = ctx.enter_context(tc.tile_pool(name="kp", bufs=nbuf))
        vpool = ctx.enter_context(tc.tile_pool(name="vp", bufs=nbuf))
        epool = ctx.enter_context(tc.tile_pool(name="ep", bufs=4))
        spool = ctx.enter_context(tc.tile_pool(name="sp", bufs=5, space="PSUM"))
        opool = ctx.enter_context(tc.tile_pool(name="op", bufs=3, space="PSUM"))
        rpool = ctx.enter_context(tc.tile_pool(name="rp", bufs=4))

        q_sb = qpool.tile([128, groups * GQ], DT)
        nc.sync.dma_start(out=q_sb, in_=q_ext[:, :])

        for g in range(groups):
            s, h = divmod(g, kv_heads)
            kt = kpool.tile([128, nchunk * 128], kdt)
            nc.sync.dma_start(out=kt, in_=kt_ext[s, h])
            vt = vpool.tile([128, nchunk, VW], DT)
            nc.scalar.dma_start(out=vt, in_=v_ext[s, h])

            ps = spool.tile([128, nchunk, GQ], f32)
            for c in range(nchunk):
                nc.tensor.matmul(
                    ps[:, c, :],
                    lhsT=kt[:, c * 128 : (c + 1) * 128],
                    rhs=q_sb[:, g * GQ : (g + 1) * GQ],
                    start=True,
                    stop=True,
                )
            et = epool.tile([128, nchunk, GQ], DT)
            nc.scalar.activation(
                out=et, in_=ps, func=mybir.ActivationFunctionType.Exp,
                scale=SCALE,
            )
            po = opool.tile([GQ, VW], f32)
            for c in range(nchunk):
                nc.tensor.matmul(
                    po[:, :],
                    lhsT=et[:, c, :],
                    rhs=vt[:, c, :],
                    start=(c == 0),
                    stop=(c == nchunk - 1),
                )
            recip = rpool.tile([GQ, 1], f32)
            nc.vector.reciprocal(out=recip, in_=po[:, HEAD_DIM:VW])
            osb = rpool.tile([GQ, HEAD_DIM], f32)
            nc.vector.tensor_scalar_mul(
                out=osb, in0=po[:, :HEAD_DIM], scalar1=recip
            )
            nc.sync.dma_start(out=out_ext[g * GQ:(g + 1) * GQ, :], in_=osb)
    nc.compile()
    return nc


def prep_core_inputs(q, k, v, k_cache, v_cache, slot_mapping, block_tables,
                     context_lens, k_fp8=True):
    """Host-side shard + layout prep. Returns (in_maps, fix_rows) where
    fix_rows maps seq index -> [NUM_HEADS*HEAD_DIM] override for degenerate
    context_len == 0 sequences (reference softmaxes all -1e30 -> uniform)."""
    np_kdt = ml_dtypes.float8_e3m4 if k_fp8 else NP_DT
    q = np.ascontiguousarray(np.asarray(q, dtype=np.float32))
    kr = np.asarray(k, dtype=np.float32).reshape(BATCH, NUM_KV_HEADS, HEAD_DIM)
    vr = np.asarray(v, dtype=np.float32).reshape(BATCH, NUM_KV_HEADS, HEAD_DIM)
    bt = np.asarray(block_tables).astype(np.int64)
    slots = np.asarray(slot_mapping).astype(np.int64)
    ctx = np.asarray(context_lens).astype(np.int64)

    # paged gather: [B, blocks_per_seq, block, kvh, dh]
    kg = np.asarray(k_cache, dtype=np.float32)[bt]
    vg = np.asarray(v_cache, dtype=np.float32)[bt]
    # scatter the new token k/v (reference scatters into the pool pre-gather,
    # so a written slot appears in every sequence whose table holds its block)
    blk, off = slots // BLOCK_SIZE, slots % BLOCK_SIZE
    for b2 in range(BATCH):
        for b, j in np.argwhere(bt == blk[b2]):
            kg[b, j, off[b2]] = kr[b2]
            vg[b, j, off[b2]] = vr[b2]
    kg = kg.reshape(BATCH, CTX, NUM_KV_HEADS, HEAD_DIM)
    vg = vg.reshape(BATCH, CTX, NUM_KV_HEADS, HEAD_DIM)

    fix_rows = {}
    for b in np.nonzero(ctx == 0)[0]:
        # all scores masked -> softmax is uniform over every key
        m = vg[b].mean(axis=0)  # [kvh, dh]
        fix_rows[int(b)] = np.repeat(m, GQ, axis=0).reshape(-1)

    valid = (np.arange(CTX)[None, :] < ctx[:, None]).astype(np.float32)  # [B,S]

    in_maps = []
    for c in range(N_CORES):
        sl = slice(c * SEQ_PER_CORE, (c + 1) * SEQ_PER_CORE)
        # K^T layout: [seq, kvh, dh, keys]
        kt_dev = np.ascontiguousarray(
            kg[sl].transpose(0, 2, 3, 1)).astype(np_kdt)
        # V + valid column, chunked: [seq, kvh, key_low(128), chunk, VW]
        vb = vg[sl] * valid[sl][:, :, None, None]      # [8, S, kvh, dh]
        va = np.empty((SEQ_PER_CORE, CTX, NUM_KV_HEADS, VW), dtype=np.float32)
        va[..., :HEAD_DIM] = vb
        va[..., HEAD_DIM] = valid[sl][:, :, None]
        v_dev = np.ascontiguousarray(
            va.reshape(SEQ_PER_CORE, NCHUNK, 128, NUM_KV_HEADS, VW)
              .transpose(0, 3, 2, 1, 4)).astype(NP_DT)
        # q^T layout: [dh, seq*kvh*gq]
        qt_dev = np.ascontiguousarray(
            q[sl].reshape(SEQ_PER_CORE, NUM_HEADS, HEAD_DIM)
                 .transpose(2, 0, 1).reshape(HEAD_DIM, -1)).astype(NP_DT)
        in_maps.append({"kt": kt_dev, "vv": v_dev, "qt": qt_dev})
    return in_maps, fix_rows


def kernel(q, k, v, k_cache, v_cache, slot_mapping, block_tables,
           context_lens):
    ctx = np.asarray(context_lens).astype(np.int64)
    # fp8 K relies on long-softmax averaging of quantization noise; with
    # short contexts fall back to bf16 K (still well under the HBM roofline).
    k_fp8 = bool(ctx.min() >= FP8_MIN_CTX)
    in_maps, fix_rows = prep_core_inputs(
        q, k, v, k_cache, v_cache, slot_mapping, block_tables, context_lens,
        k_fp8=k_fp8)
    key = "fp8" if k_fp8 else "bf16"
    if key not in _NC_CACHE:
        _NC_CACHE[key] = build_nc(k_fp8=k_fp8)
    nc = _NC_CACHE[key]
    res = run_bass_kernel_spmd(nc, in_maps, list(range(N_CORES))).results
    out = np.empty((BATCH, NUM_HEADS * HEAD_DIM), dtype=np.float32)
    for c in range(N_CORES):
        # rows are s*32 + kvh*4 + j == s*32 + h_q (repeat_interleave order)
        out[c * SEQ_PER_CORE:(c + 1) * SEQ_PER_CORE] = (
            res[c]["out"].reshape(SEQ_PER_CORE, NUM_HEADS * HEAD_DIM))
    for b, row in fix_rows.items():
        out[b] = row
    return out

